# revision 8
# baseline (speedup 1.0000x reference)
"""KAN layer (nn_KANLayer) on 8 Trainium2 NeuronCores — Bass kernel.

Math: reference computes out[b,j] = sum_{i,k} basis_k(tanh(x[b,i])) * C[j,i,k]
where basis_k is a linear "hat" on knots [t_k, t_{k+1}, t_{k+2}] (k = 0..11;
basis column 12 is always zero).

Device-side we use "min-ramp" features (one fused tensor_scalar each):

    M_m(xc) = min(xc - t_m, 0),   m = 1..13   (M_0 == 0 since xc >= -1)

The hat is the second difference of min-ramps, folded into host-side weights:

    -hat_k/denom_k = (M_k - 2*M_{k+1} + M_{k+2}) * s_k
    out[b,j] = sum_m sum_i M_m(xc[b,i]) * D_m[j,i]

with D_m[j,i] = -(C'_m - 2 C'_{m-1} + C'_{m-2}),  C'_k = C[:,:,k] * s_k.

This is a dense matmul with contraction (i,m) = 1024*13, run in f32r
(full-rate fp32 on the PE, 4x faster than plain fp32).  Sharding:
data-parallel over batch (8192 -> 8 x 1024), weights replicated; x is
pre-transposed on host so tiles are [i_partition, b_free]; the output comes
back [j, b] per core and is transposed on host.
"""
import contextlib

import numpy as np

import concourse.bass as bass
import concourse.mybir as mybir
from concourse import bass_utils

F32 = mybir.dt.float32
F32R = mybir.dt.float32r

B, I, J, NB = 8192, 1024, 256, 13
NCORES = 8
BLOC = B // NCORES          # 1024 batch rows per core
NM = 13                     # min-ramp features m = 1..13
NIT = I // 128              # 8 i-tiles
EPS = 1e-8

DVE_MS = list(range(0, 7))   # feature indices (0-based into the 13)
GPS_MS = list(range(7, 10))
ACT_MS = list(range(10, 13))  # computed as relu(t_m - xc) = -M_m (sign in weights)

_cached = None


def _knots64():
    return np.linspace(-1.0, 1.0, 16).astype(np.float32).astype(np.float64)


def _build():
    kn = _knots64()
    # thresholds for features: t_1 .. t_13 (fp32 values)
    thr = [float(np.float32(kn[m])) for m in range(1, 14)]

    nc = bass.Bass("TRN2", target_bir_lowering=False, debug=False,
                   num_devices=NCORES)

    xd = nc.declare_dram_parameter("x", [I, BLOC], F32, isOutput=False)
    wd = nc.declare_dram_parameter("w", [NIT, 128, NM * J], F32R, isOutput=False)
    yd = nc.declare_dram_parameter("y", [J, BLOC], F32, isOutput=True)

    # const APs for ACT feature biases (t_m values)
    for mi in ACT_MS:
        cval = thr[mi]
        t = nc.alloc_sbuf_tensor(f"const-thr-{mi}", [128, 1], F32)
        nc.gpsimd.memset(t.ap(), cval)
        nc.const_aps.aps[(F32, cval)] = t.ap()
    nc.all_engine_barrier()

    ctx = contextlib.ExitStack()
    xbuf = [ctx.enter_context(nc.sbuf_tensor(f"xbuf{p}", [128, BLOC], F32))
            for p in range(2)]
    xcb = [ctx.enter_context(nc.sbuf_tensor(f"xcb{p}", [128, BLOC], F32))
           for p in range(2)]
    fbuf = [[ctx.enter_context(nc.sbuf_tensor(f"fbuf{m}_{p}", [128, BLOC], F32R))
             for p in range(2)] for m in range(NM)]
    wbuf = [ctx.enter_context(nc.sbuf_tensor(f"wbuf{p}", [128, NM * J], F32R))
            for p in range(2)]
    ps = [[ctx.enter_context(nc.psum_tensor(f"ps{jh}_{bh}", [128, 512], F32))
           for bh in range(2)] for jh in range(2)]
    obuf = [ctx.enter_context(nc.sbuf_tensor(f"obuf{jh}", [128, BLOC], F32))
            for jh in range(2)]

    with ctx:
        with (
            nc.semaphore() as s_x,
            nc.semaphore() as s_w,
            nc.semaphore() as s_xc,
            nc.semaphore() as s_fdv,
            nc.semaphore() as s_fdg,
            nc.semaphore() as s_fda,
            nc.semaphore() as s_pe,
            nc.semaphore() as s_cp,
            nc.Block() as block,
        ):
            @block.sync
            def _(sync):
                WH = (NM * J) // 2
                for i in range(NIT):
                    p = i % 2
                    if i >= 2:
                        sync.wait_ge(s_xc, i - 1)
                    sync.dma_start(out=xbuf[p][:], in_=xd[i * 128:(i + 1) * 128, :]
                                   ).then_inc(s_x, 16)
                    if i >= 2:
                        sync.wait_ge(s_pe, i - 1)
                    sync.dma_start(out=wbuf[p][:, :WH],
                                   in_=wd[i][:, :WH]).then_inc(s_w, 16)
                sync.wait_ge(s_cp, 4)
                for jh in range(2):
                    sync.dma_start(out=yd[jh * 128:(jh + 1) * 128, :],
                                   in_=obuf[jh][:]).then_inc(s_x, 16)

            @block.scalar
            def _(scalar):
                WH = (NM * J) // 2
                for i in range(NIT):
                    p = i % 2
                    if i >= 2:
                        scalar.wait_ge(s_pe, i - 1)
                    scalar.dma_start(out=wbuf[p][:, WH:],
                                     in_=wd[i][:, WH:]).then_inc(s_w, 16)
                    scalar.wait_ge(s_x, 16 * (i + 1))
                    if i >= 2:
                        # xc[p] must be fully consumed by tile i-2's features
                        if DVE_MS:
                            scalar.wait_ge(s_fdv, i - 1)
                        if GPS_MS:
                            scalar.wait_ge(s_fdg, i - 1)
                    nc.scalar.activation(xcb[p][:], xbuf[p][:],
                                         mybir.ActivationFunctionType.Tanh
                                         ).then_inc(s_xc, 1)
                    last = None
                    for m in ACT_MS:
                        # fbuf = relu(t_m - xc) = -M_m
                        last = nc.scalar.activation(
                            fbuf[m][p][:], xcb[p][:],
                            mybir.ActivationFunctionType.Relu,
                            bias=thr[m], scale=-1.0)
                    last.then_inc(s_fda, 1)

            def feature_prog(engine_ap, ms, sem):
                def prog(eng):
                    for i in range(NIT):
                        p = i % 2
                        eng.wait_ge(s_xc, i + 1)
                        if i >= 2:
                            eng.wait_ge(s_pe, i - 1)
                        last = None
                        for m in ms:
                            last = engine_ap.tensor_scalar(
                                fbuf[m][p][:], xcb[p][:], thr[m], 0.0,
                                mybir.AluOpType.subtract, mybir.AluOpType.min)
                        last.then_inc(sem, 1)
                return prog

            if DVE_MS:
                block.vector(feature_prog(nc.vector, DVE_MS, s_fdv))
            if GPS_MS:
                block.gpsimd(feature_prog(nc.gpsimd, GPS_MS, s_fdg))

            @block.tensor
            def _(tensor):
                for i in range(NIT):
                    p = i % 2
                    tensor.wait_ge(s_w, 32 * (i + 1))
                    if DVE_MS:
                        tensor.wait_ge(s_fdv, i + 1)
                    last = None
                    gps_waited = False
                    act_waited = False
                    for m in DVE_MS + GPS_MS + ACT_MS:
                        if (not gps_waited) and m in GPS_MS:
                            tensor.wait_ge(s_fdg, i + 1)
                            gps_waited = True
                        if (not act_waited) and m in ACT_MS:
                            tensor.wait_ge(s_fda, i + 1)
                            act_waited = True
                        for jh in range(2):
                            lhsT = wbuf[p][:, (m * 2 + jh) * 128:(m * 2 + jh + 1) * 128]
                            for bh in range(2):
                                last = nc.tensor.matmul(
                                    ps[jh][bh][:],
                                    lhsT,
                                    fbuf[m][p][:, bh * 512:(bh + 1) * 512],
                                    start=(i == 0 and m == DVE_MS[0]),
                                    stop=(i == NIT - 1 and m == ACT_MS[-1]),
                                )
                    last.then_inc(s_pe, 1)

            # PSUM -> SBUF copies after all matmuls: 2 on DVE, 2 on scalar
            @block.vector
            def _(vector):
                vector.wait_ge(s_pe, NIT)
                for bh in range(2):
                    nc.vector.tensor_copy(obuf[0][:, bh * 512:(bh + 1) * 512],
                                          ps[0][bh][:]).then_inc(s_cp, 1)

            @block.scalar
            def _(scalar):
                scalar.wait_ge(s_pe, NIT)
                for bh in range(2):
                    nc.scalar.copy(obuf[1][:, bh * 512:(bh + 1) * 512],
                                   ps[1][bh][:]).then_inc(s_cp, 1)

    return nc


def _weights(spline_coeffs, knots=None):
    """W[it, i_local, (m-1)*J + j] = D_m[j, it*128 + i_local],  m = 1..13."""
    kn = _knots64() if knots is None else np.asarray(knots, np.float32).astype(np.float64)
    C = spline_coeffs.astype(np.float64)          # [J, I, NB]
    s = np.array([0.5 * (1.0 / (kn[k + 1] - kn[k] + EPS)
                         + 1.0 / (kn[k + 2] - kn[k + 1] + EPS))
                  for k in range(12)])
    Cp = C[:, :, :12] * s[None, None, :]
    W = np.zeros((I, NM, J), dtype=np.float64)    # index 0 -> m=1
    for mi in range(NM):
        m = mi + 1
        acc = np.zeros((J, I))
        if m <= 11:
            acc += Cp[:, :, m]
        if 0 <= m - 1 <= 11:
            acc -= 2.0 * Cp[:, :, m - 1]
        if 0 <= m - 2 <= 11:
            acc += Cp[:, :, m - 2]
        W[:, mi, :] = acc.T if mi in ACT_MS else -acc.T
    W = W.reshape(NIT, 128, NM * J)
    return np.ascontiguousarray(W, dtype=np.float32)


def kernel(x, spline_coeffs, knots):
    global _cached
    x = np.asarray(x, dtype=np.float32)
    spline_coeffs = np.asarray(spline_coeffs, dtype=np.float32)

    if _cached is None:
        _cached = _build()
    nc = _cached

    Wf = _weights(spline_coeffs, knots)
    in_maps = []
    for c in range(NCORES):
        xT = np.ascontiguousarray(x[c * BLOC:(c + 1) * BLOC, :].T)  # [I, BLOC]
        in_maps.append({"x": xT, "w": Wf})

    res = bass_utils.run_bass_kernel_spmd(nc, in_maps,
                                          core_ids=list(range(NCORES)))
    out = np.empty((B, J), dtype=np.float32)
    for c in range(NCORES):
        out[c * BLOC:(c + 1) * BLOC, :] = res.results[c]["y"].T
    return out
